# revision 47
# baseline (speedup 1.0000x reference)
"""Trainium2 Bass kernel for nn_Encoder_conv_mlp (GNN message passing encoder).

Reference computation (per graph batch):
    h1 = relu(segsum(x[src]->dst) @ W1_rel.T + x @ W1_root.T + b1)
    h2 = relu(segsum(h1[src]->dst) @ W2_rel.T + h1 @ W2_root.T + b2)
    hb = h2.reshape(bs, 64*256)
    mu = hb @ Wmu.T + bmu ; logvar = hb @ Wlv.T + blv

Sharding: data-parallel over graphs. 512 graphs / 8 cores = 64 graphs
(4096 nodes, 65536 edges) per core. Edges never cross graphs, so each
core is fully independent; weights are replicated and the host simply
concatenates the per-core [64, 256] outputs.

Message passing is done as dense matmuls: the host builds, for every
2-graph block (128 nodes), an adjacency count matrix A2T[s, d] =
#edges(src=s -> dst=d). On device, aggregation is A2T contracted over
the src-node partition dim. Two matmul "families" avoid all transposes:
  - activations stationary (lhsT) + weights moving  -> node-major out
  - weights stationary (lhsT) + activations moving  -> feature-major out
Layer outputs are kept feature-major; the rel-projection (node-major) is
an intermediate only. All matmul operands are bf16 (fp32 PSUM accum).

The [16384, 256] readout weight [Wmu.T | Wlv.T] (8.4 MB bf16 per core)
is prefetched into SBUF while the conv layers run. Inputs are loaded
into per-chunk SBUF tiles so compute starts as soon as its chunk lands
(whole-tile deps would stall the PE on the full transfer), and h1 is
split per (ko, group) so layer 2 pipelines behind layer 1.

Further scheduling details: the f32 biases and the w1 weights ride
packed inside the xw/w2 bf16 input tensors (f32 values via bitcast
views on device) so the serial per-DMA launch overhead is paid fewer
times and the first matmul's dependencies arrive in a single transfer;
a short stream of discarded warm-up matmuls keeps the PE clock ramp
(HAM) busy while the first input DMAs land; and layer 2 runs all
rel-projections first, then the whole mo=0 output pass before mo=1, so
h2's first feature half (which gates the readout) completes while the
PE still has a full pass of work queued.
"""
import sys

if "/opt/trn_rl_repo" not in sys.path:
    sys.path.insert(0, "/opt/trn_rl_repo")

import numpy as np
import ml_dtypes

N_NODES = 64
BS = 512
IN_F = 128
HID = 256
LAT = 128
N_CORES = 8
G_PER = BS // N_CORES          # 64 graphs per core
NODES_PER = G_PER * N_NODES    # 4096 nodes per core
BLOCKS = NODES_PER // 128      # 32 two-graph blocks per core
GROUPS = NODES_PER // 512      # 8 512-node groups per core
KT = (N_NODES * HID) // 128    # 128 readout contraction tiles

BF16 = ml_dtypes.bfloat16

_PROGRAM = None


def _build_program():
    import concourse.bacc as bacc
    import concourse.mybir as mybir
    import concourse.tile as tile

    nc = bacc.Bacc("TRN2", target_bir_lowering=False, debug=False,
                   num_devices=N_CORES)
    BF = mybir.dt.bfloat16
    F32 = mybir.dt.float32

    xw = nc.dram_tensor("xw", [128, 520 + NODES_PER], BF,
                        kind="ExternalInput").ap()
    a2t = nc.dram_tensor("a2t", [128, BLOCKS * 128], BF, kind="ExternalInput").ap()
    w2 = nc.dram_tensor("w2", [128, 1536], BF, kind="ExternalInput").ap()
    wro = nc.dram_tensor("wro", [128, KT * 256], BF, kind="ExternalInput").ap()
    out = nc.dram_tensor("out", [G_PER, 256], F32, kind="ExternalOutput").ap()

    Relu = mybir.ActivationFunctionType.Relu

    with tile.TileContext(nc) as tc:
        with (
            tc.tile_pool(name="const", bufs=1) as const,
            tc.tile_pool(name="hr", bufs=20) as hr_pool,
            tc.tile_pool(name="psum_hr", bufs=3, space="PSUM") as psum_hr,
            tc.tile_pool(name="psum_fm", bufs=4, space="PSUM") as psum_fm,
            tc.tile_pool(name="psum_ro", bufs=1, space="PSUM") as psum_ro,
        ):
            # Per-chunk tiles so each consumer depends only on its chunk's DMA.
            lead_sb = const.tile([128, 1032], BF, tag="lead_sb")
            xT0b_sb = const.tile([128, 512], BF, tag="xT0b_sb")
            xT_sb = [const.tile([128, 1024], BF, name=f"xT{i}", tag=f"xT{i}")
                     for i in range(1, 4)]
            a2t_sb = [const.tile([128, 1024], BF, name=f"a2t{i}", tag=f"a2t{i}")
                      for i in range(4)]
            w2_sb = const.tile([128, 1536], BF, tag="w2_sb")
            wro_sb = [const.tile([128, 4096], BF, name=f"wro{i}", tag=f"wro{i}") for i in range(8)]
            # h1 split per (ko, group) for L1->L2 pipelining; h2 per ko chunk.
            h1_sb = [[const.tile([128, 512], BF, name=f"h1_{ko}_{g}", tag=f"h1_{ko}_{g}")
                      for g in range(GROUPS)] for ko in range(2)]
            h2_sb = [const.tile([128, NODES_PER], BF, name=f"h2_{fo}", tag=f"h2_{fo}")
                     for fo in range(2)]

            # DMA issue order = priority order for the head of the kernel.
            # The lead transfer carries w1 + biases + the first node group's
            # x in one launch (the first matmul's full dependency set);
            # a2t0 follows for the first aggregation, then x/a2t chunks
            # interleave in consumption order ahead of w2 and the big
            # readout-weight stream.
            nc.sync.dma_start(lead_sb[:], xw[:, 0:1032])
            nc.sync.dma_start(a2t_sb[0][:], a2t[:, 0:1024])
            nc.sync.dma_start(xT0b_sb[:], xw[:, 1032:1544])
            for i in range(1, 4):
                nc.sync.dma_start(xT_sb[i - 1][:],
                                  xw[:, 520 + i * 1024:520 + (i + 1) * 1024])
                nc.sync.dma_start(a2t_sb[i][:], a2t[:, i * 1024:(i + 1) * 1024])
            nc.sync.dma_start(w2_sb[:], w2[:])
            # w1 + biases ride packed inside lead/w2 (bitcast views for f32)
            w1_sb = lead_sb[:, 0:520]
            b12_sb = lead_sb[:, 512:520].bitcast(F32)
            bout_sb = w2_sb[0:64, 1024:1536].bitcast(F32)
            for i in range(8):
                nc.sync.dma_start(wro_sb[i][:], wro[:, i * 4096:(i + 1) * 4096])

            # PE pre-warm: dummy matmuls on memset data keep the PE busy from
            # ~0.7us so the clock ramp (HAM) completes before the first real
            # matmul arrives behind the input DMAs (~3.9us). Results are
            # discarded; the psum slot is reused by the readout much later.
            N_WARM = 13
            ones_sb = const.tile([1, 320], BF, tag="ones_sb")
            nc.gpsimd.memset(ones_sb[:], 1.0)
            warm = psum_ro.tile([G_PER, 256], F32, tag="pro")
            for i in range(N_WARM):
                nc.tensor.matmul(warm[:], lhsT=ones_sb[:, 256:320],
                                 rhs=ones_sb[:, 0:256],
                                 start=(i == 0), stop=(i == N_WARM - 1))

            def x_cols(c0, c1):        # feature-major x slice [128, c1-c0]
                g = c0 // 512
                if g == 0:
                    assert c1 <= 512
                    return lead_sb[:, 520 + c0:520 + c1]
                if g == 1:
                    assert c1 <= 1024
                    return xT0b_sb[:, c0 - 512:c1 - 512]
                i = (c0 - 1024) // 1024
                assert c1 - 1024 <= (i + 1) * 1024
                return xT_sb[i][:, c0 - 1024 - i * 1024:c1 - 1024 - i * 1024]

            def a2t_blk(b):            # [128, 128] adjacency for block b
                i = b // 8
                return a2t_sb[i][:, (b % 8) * 128:(b % 8 + 1) * 128]

            # ---- Conv layers ----
            for layer in range(2):
                n_ko = 1 if layer == 0 else 2
                if layer == 0:
                    act_cols = lambda ko, c0, c1: x_cols(c0, c1)
                    w_rel = lambda ko: w1_sb[:, 0:256]
                    w_root = lambda ko, mo: w1_sb[:, 256 + mo * 128:
                                                  256 + (mo + 1) * 128]
                    bias_col = 0
                else:
                    act_cols = lambda ko, c0, c1: (
                        h1_sb[ko][c0 // 512][:, c0 % 512:c0 % 512 + (c1 - c0)])
                    w_rel = lambda ko: w2_sb[:, ko * 512:ko * 512 + 256]
                    w_root = lambda ko, mo: w2_sb[:, ko * 512 + 256 + mo * 128:
                                                  ko * 512 + 256 + (mo + 1) * 128]
                    bias_col = 2

                def emit_hr(grp):
                    # two blocks share one [128,512] psum tile (same bank
                    # footprint as a padded [128,256]) so one DVE copy evicts
                    # both -> half the copy count, ~4us less DVE busy
                    hrs = []
                    for pair in range(2):
                        ph = psum_hr.tile([128, 512], F32)
                        for sub in range(2):
                            b = grp * 4 + pair * 2 + sub
                            for ko in range(n_ko):
                                nc.tensor.matmul(
                                    ph[:, sub * 256:(sub + 1) * 256],
                                    lhsT=act_cols(ko, b * 128, (b + 1) * 128),
                                    rhs=w_rel(ko),
                                    start=(ko == 0), stop=(ko == n_ko - 1),
                                    skip_group_check=True,
                                )
                        hr = hr_pool.tile([128, 512], BF)
                        nc.vector.tensor_copy(hr[:], ph[:])
                        hrs.append(hr)
                    return hrs

                def emit_fm(grp, mo, hrs):
                    pf = psum_fm.tile([128, 512], F32, name="pf", tag="pf")
                    for ko in range(n_ko):
                        nc.tensor.matmul(
                            pf[:],
                            lhsT=w_root(ko, mo),
                            rhs=act_cols(ko, grp * 512, (grp + 1) * 512),
                            start=(ko == 0), stop=False,
                            skip_group_check=True,
                        )
                    for blk in range(4):
                        b = grp * 4 + blk
                        nc.tensor.matmul(
                            pf[:, blk * 128:(blk + 1) * 128],
                            lhsT=hrs[blk // 2][:, (blk % 2) * 256 + mo * 128:
                                               (blk % 2) * 256 + (mo + 1) * 128],
                            rhs=a2t_blk(b),
                            start=False, stop=(blk == 3),
                            skip_group_check=True,
                        )
                    if layer == 0:
                        dst = h1_sb[mo][grp][:]
                    else:
                        dst = h2_sb[mo][:, grp * 512:(grp + 1) * 512]
                    nc.scalar.activation(
                        dst, pf[:], Relu,
                        bias=b12_sb[:, bias_col + mo:bias_col + mo + 1],
                    )

                if layer == 0:
                    for grp in range(GROUPS):
                        hrs = emit_hr(grp)
                        for mo in range(2):
                            emit_fm(grp, mo, hrs)
                else:
                    # L2: all hr projections first, then the whole mo=0 pass
                    # before mo=1 — h2_sb[0] (which gates the readout's fo=0
                    # k-tiles) completes while the PE still has the entire
                    # mo=1 pass queued, hiding the readout-start stall.
                    all_hrs = [emit_hr(grp) for grp in range(GROUPS)]
                    for mo in range(2):
                        for grp in range(GROUPS):
                            emit_fm(grp, mo, all_hrs[grp])

            # ---- Readout ----
            # out[g, l] += sum_f h2_fm[fo][f, g*64+n] * wro[kt*128+f, l]
            pro = psum_ro.tile([G_PER, 256], F32, tag="pro")
            # fo=0 k-tiles first: the readout then only waits on h2_sb[0],
            # whose last eviction lands one ACT-op earlier than h2_sb[1]'s.
            kts = [kt for kt in range(KT) if kt % 2 == 0] + \
                  [kt for kt in range(KT) if kt % 2 == 1]
            for i, kt in enumerate(kts):
                n, fo = kt // 2, kt % 2
                lhsT = h2_sb[fo][:, n:n + (G_PER - 1) * N_NODES + 1:N_NODES]
                nc.tensor.matmul(
                    pro[:], lhsT=lhsT,
                    rhs=wro_sb[kt // 16][:, (kt % 16) * 256:(kt % 16 + 1) * 256],
                    start=(i == 0), stop=(i == KT - 1),
                )
            out_sb = const.tile([G_PER, 256], F32, tag="out_sb")
            nc.vector.tensor_add(out_sb[:], pro[:], bout_sb[:])
            nc.sync.dma_start(out[:], out_sb[:])

    nc.compile()
    return nc


def _get_program():
    global _PROGRAM
    if _PROGRAM is None:
        _PROGRAM = _build_program()
    return _PROGRAM


def make_in_maps(x, W1_rel, W1_root, b1, W2_rel, W2_root, b2,
                 Wmu, bmu, Wlv, blv, edge_index, batch):
    """Host-side shard + layout prep. Returns per-core input dicts."""
    x = np.asarray(x, dtype=np.float32)
    edge_index = np.asarray(edge_index)

    b12 = np.stack(
        [np.asarray(b1)[0:128], np.asarray(b1)[128:256],
         np.asarray(b2)[0:128], np.asarray(b2)[128:256]], axis=1
    ).astype(np.float32)
    w1_pack = np.concatenate(
        [np.concatenate([np.asarray(W1_rel).T, np.asarray(W1_root).T],
                        axis=1).astype(BF16),
         np.ascontiguousarray(b12).view(BF16)], axis=1)
    w2rT = np.asarray(W2_rel).T.astype(np.float32)
    w2tT = np.asarray(W2_root).T.astype(np.float32)
    bout = np.broadcast_to(
        np.concatenate([np.asarray(bmu), np.asarray(blv)])[None, :],
        (G_PER, 256)).astype(np.float32)
    bout_pack = np.zeros((128, 512), BF16)
    bout_pack[0:G_PER] = np.ascontiguousarray(bout).view(BF16)
    w2 = np.concatenate(
        [np.concatenate([w2rT[0:128], w2tT[0:128]], axis=1).astype(BF16),
         np.concatenate([w2rT[128:256], w2tT[128:256]], axis=1).astype(BF16),
         bout_pack], axis=1)
    wro_cat = np.concatenate([np.asarray(Wmu).T, np.asarray(Wlv).T], axis=1)
    wro = np.ascontiguousarray(
        wro_cat.reshape(KT, 128, 256).transpose(1, 0, 2).reshape(128, KT * 256)
    ).astype(BF16)

    # Dense per-2-graph-block adjacency counts: A[blk][s, d] = #edges s->d.
    src = edge_index[0].astype(np.int64)
    dst = edge_index[1].astype(np.int64)
    blk = dst >> 7                       # 128 nodes per 2-graph block
    s_loc = src - (blk << 7)
    d_loc = dst - (blk << 7)
    # edges are intra-graph by construction; fail loudly rather than let a
    # cross-block index wrap around in np.add.at
    assert s_loc.min() >= 0 and s_loc.max() < 128, "edge crosses graph block"
    A = np.zeros((BS // 2, 128, 128), np.float32)
    np.add.at(A, (blk, s_loc, d_loc), 1.0)

    in_maps = []
    for c in range(N_CORES):
        xs = x[c * NODES_PER:(c + 1) * NODES_PER]
        xw = np.concatenate(
            [w1_pack, np.ascontiguousarray(xs.T).astype(BF16)], axis=1)
        Ac = A[c * BLOCKS:(c + 1) * BLOCKS]
        a2t = np.ascontiguousarray(
            Ac.transpose(1, 0, 2).reshape(128, BLOCKS * 128)
        ).astype(BF16)
        in_maps.append(dict(xw=xw, a2t=a2t, w2=w2, wro=wro))
    return in_maps


def kernel(**inputs):
    from concourse.bass_utils import run_bass_kernel_spmd

    nc = _get_program()
    in_maps = make_in_maps(**inputs)
    res = run_bass_kernel_spmd(nc, in_maps, list(range(N_CORES)))
    outs = np.concatenate(
        [res.results[c]["out"] for c in range(N_CORES)], axis=0)  # [512, 256]
    mu = np.ascontiguousarray(outs[:, :LAT]).astype(np.float32)
    logvar = np.ascontiguousarray(outs[:, LAT:]).astype(np.float32)
    return mu, logvar

